# revision 1
# baseline (speedup 1.0000x reference)
"""Trainium2 Bass kernel for nn_CategoryMultiplier.

out[b, s, :] = inputs[b, s, :] * (emb_table[categories[b, s]] if
               categories[b, s] != 0 else 1.0)

Sharding: pure data parallel over batch. 8 cores x 16 batches each.
Per core: x flat [8192, 512] f32, cats (int16 permuted + wrapped for
dma_gather), table [1000, 512] f32.

Device layout: positions are partition-major (partition p holds positions
p*64 .. p*64+63) so the input/output DMAs use 16KB-contiguous descriptors
per partition (HWDGE emission is ~6ns/descriptor on the issuing engine).
Embedding rows are fetched with one InstDMAGatherAnt per chunk
(~10ns/row of Q7 time); its fixed dst layout dst[i%128, i//128] is
reconciled with the partition-major layout by permuting the index array
on the host (pure layout prep).

Padding (category 0 -> multiplier 1.0): row 0 of the table is
semantically dead (index 0 always masks to 1.0), so the kernel overwrites
row 0 of the device-side table buffer in place with ones from an inline
const tensor - a zero-copy stand-in for an all-ones-row table. Row 0 is
then only ever gathered by padding positions.
"""

import numpy as np

import concourse.bass as bass
import concourse.bacc as bacc
import concourse.mybir as mybir
import concourse.tile as tile
from concourse.bass_utils import run_bass_kernel_spmd

# Problem shape (hardcoded per harness contract).
B, S, D = 128, 512, 512
VOCAB = 1000
N_CORES = 8
B_LOC = B // N_CORES            # 16 batches per core
N = B_LOC * S                   # 8192 positions per core
P = 128                         # SBUF partitions
C = N // P                      # 64 positions per partition
T_CH = 8                        # max positions-per-partition per chunk

F32 = mybir.dt.float32
I16 = mybir.dt.int16

# Taper: small chunks at head (prime the pipeline) and tail (short drain).
CHUNKS = [4, 4] + [8] * 6 + [4, 4]
assert sum(CHUNKS) == C


def _build_nc():
    nc = bacc.Bacc("TRN2", target_bir_lowering=False, debug=False)

    x = nc.dram_tensor("x", [N, D], F32, kind="ExternalInput")
    cats16 = nc.dram_tensor("cats16", [P, N // 16], I16, kind="ExternalInput")
    table = nc.dram_tensor("table", [VOCAB, D], F32, kind="ExternalInput")
    y = nc.dram_tensor("y", [N, D], F32, kind="ExternalOutput")

    xr = x[:].rearrange("(p c) d -> p (c d)", p=P)     # [128, C*D]
    yr = y[:].rearrange("(p c) d -> p (c d)", p=P)

    # Issue the GPSIMD ucode library load BEFORE the TileContext so the
    # ~14us IRAM load overlaps Tile's own prologue barrier instead of
    # running after it.
    from concourse.library_config import mlp
    nc.gpsimd.load_library(mlp)

    with tile.TileContext(nc) as tc:
        with (
            tc.tile_pool(name="const", bufs=1) as const_pool,
            tc.tile_pool(name="io", bufs=5) as io_pool,
            tc.tile_pool(name="gat", bufs=6) as gat_pool,
        ):
            # Tiny prerequisites on the ACT ring. The ones row comes from an
            # inline const tensor (materialized at model load), so no
            # gpsimd memset competes with the library load on Q7.
            cats_t = const_pool.tile([P, N // 16], I16)
            nc.scalar.dma_start(out=cats_t[:], in_=cats16[:])
            ones_dram = nc.inline_tensor(np.ones((1, D), dtype=np.float32),
                                         name="ones_row")
            nc.scalar.dma_start(out=table[0:1, :], in_=ones_dram[:])

            pos = 0
            for ci, tch in enumerate(CHUNKS):
                lo, hi = pos * D, (pos + tch) * D
                n_idx = tch * P
                g_t = gat_pool.tile([P, T_CH * D], F32, tag="g")
                nc.gpsimd.dma_gather(
                    out_ap=g_t[:, :tch * D].rearrange("p (t d) -> p t d", t=tch),
                    in_ap=table[:],
                    idxs_ap=cats_t[:, pos * 8:(pos + tch) * 8],
                    num_idxs=n_idx,
                    num_idxs_reg=n_idx,
                    elem_size=D,
                )

                x_t = io_pool.tile([P, T_CH * D], F32, tag="x")
                nc.sync.dma_start(out=x_t[:, :tch * D], in_=xr[:, lo:hi])

                nc.vector.tensor_mul(out=g_t[:, :tch * D], in0=g_t[:, :tch * D],
                                     in1=x_t[:, :tch * D])
                nc.scalar.dma_start(out=yr[:, lo:hi], in_=g_t[:, :tch * D])
                pos += tch

    nc.compile()
    return nc


_NC = None


def _get_nc():
    global _NC
    if _NC is None:
        _NC = _build_nc()
    return _NC


def _permute_cats(c):
    """Build the dma_gather index stream for the partition-major layout.

    Stream index s = col*128 + p (col = global position-per-partition)
    must hold cats[p*C + col]. Wrap (index s at [s%16, s//16]) and
    replicate across the 8 16-partition groups.
    """
    a = np.ascontiguousarray(c.reshape(P, C).T).reshape(N)   # [col, p] flat
    return np.ascontiguousarray(np.tile(a.reshape(N // 16, 16).T, (8, 1)))


def _shard_inputs(inputs, categories, emb_table):
    tab = np.ascontiguousarray(emb_table, dtype=np.float32)
    in_maps = []
    for i in range(N_CORES):
        xs = np.ascontiguousarray(
            inputs[i * B_LOC:(i + 1) * B_LOC], dtype=np.float32
        ).reshape(N, D)
        c = categories[i * B_LOC:(i + 1) * B_LOC].reshape(N).astype(np.int16)
        in_maps.append({"x": xs, "cats16": _permute_cats(c), "table": tab})
    return in_maps


def kernel(inputs, categories, mask_positions=None, emb_table=None, **_):
    """Full (unsharded) inputs in, full output out. mask_positions unused."""
    nc = _get_nc()
    in_maps = _shard_inputs(inputs, categories, emb_table)
    res = run_bass_kernel_spmd(nc, in_maps, list(range(N_CORES)))
    out = np.empty((B, S, D), dtype=np.float32)
    for i in range(N_CORES):
        out[i * B_LOC:(i + 1) * B_LOC] = res.results[i]["y"].reshape(B_LOC, S, D)
    return out

